# revision 6
# baseline (speedup 1.0000x reference)
"""Masked fractional Hamming distance over 31 circular rotations, on 8 trn2 cores.

Math: for shift s, num(s)/den(s) with
  den(s) = sum_{t,k} ma_k[t] * mb_k[t+s]          (correlation at lag s)
  num(s) = masked differing bits; with the sign-encode
  A = (ia<<7)|ma, B = (ib<<7)|mb read as fp8e4m3 the bytes become
  {+0, -0, +2^-9, -2^-9} (sign=iris, magnitude=mask), so
  corr(A, B)(s) = (den - 2*num) * 2^-18, corr(A&1, B&1)(s) = den * 2^-18.

The encode happens on the HOST (halves HBM traffic: 2 tensors instead of 4);
the two k-planes are de-interleaved on the host and summed inside the PE via
fp8 DoubleRow matmuls (lhsT [K,2,128], rhs [K,2,158] -> psum [128,158] with
result = sum_i W_i.T @ X_i at 2x fp8 rate). The +-15 circular halo is baked
into B on the host. Masks (byte&1) are extracted on-device with one u16 AND.

The plane is cut into "pieces" (a few 128-chunk windows each); each piece is
a separate DMA so compute chases the DMA stream.  Input DMAs alternate
between the SP and Activation DGE queues so descriptor issue isn't
serialized on one sequencer.  A short junk-matmul chain at kernel start
keeps the PE busy while the first piece is in flight so the HAM clock gate
reaches 2.4 GHz before the real matmul stream begins.  The last piece is
small so the PE tail after the final DMA is short.  Band diagonals are
summed on the host (exact integers scaled by 2^-18).
"""

import numpy as np

N_CORES = 8
B_FULL, L = 4096, 2048
R = 15
B_SH = B_FULL // N_CORES       # 512 batches per core
ROWS = 2 * B_SH                # 1024 rows per core (2 eyes x 512 batches)
NW = 128 + 2 * R               # 158 moving window
N_GROUPS = ROWS // 128         # 8
N_CHUNKS = L // 128            # 16
N_WARM = 14                    # junk matmuls to warm the PE clock gate


def _pad16(x):
    return -(-x // 16) * 16


# pieces: (group, first chunk, last chunk inclusive); last piece small so the
# PE tail after the final DMA packet is short
_PIECES = []
for _g in range(N_GROUPS):
    if _g < N_GROUPS - 1:
        _PIECES += [(_g, 0, 8), (_g, 9, 15)]
    else:
        _PIECES += [(_g, 0, 8), (_g, 9, 12), (_g, 13, 15)]


def _piece_geom(c0, c1):
    """A-width, B-width, padded plane stride for chunks [c0, c1]."""
    a_w = (c1 - c0 + 1) * 128
    b_w = a_w + 2 * R
    return a_w, b_w, _pad16(a_w + b_w)


_CACHE = {}


def _build_program():
    import concourse.bass as bass
    import concourse.tile as tile
    from concourse import mybir

    u8 = mybir.dt.uint8
    u16 = mybir.dt.uint16
    f8 = mybir.dt.float8e4
    f32 = mybir.dt.float32
    Alu = mybir.AluOpType
    DR = mybir.MatmulPerfMode.DoubleRow

    nc = bass.Bass()
    piece_d = []
    for i, (g, c0, c1) in enumerate(_PIECES):
        _, _, w = _piece_geom(c0, c1)
        piece_d.append(
            nc.declare_dram_parameter(f"p{i}", [128, 2, w], u8, isOutput=False)
        )
    out_d = nc.declare_dram_parameter("out", [128, 2, NW], f32, isOutput=True)

    with tile.TileContext(nc) as tc:
        with (
            tc.tile_pool(name="raw", bufs=5) as raw_pool,
            tc.tile_pool(name="acc", bufs=1, space="PSUM") as psum_pool,
        ):
            ps_ab = psum_pool.tile([128, NW], f32)
            ps_mm = psum_pool.tile([128, NW], f32)

            # PE warm-up: the HAM clock gate keeps the PE at 1.2 GHz until it
            # has been busy for a full ~3.4us activity window.  The first
            # input piece takes ~3.5us to arrive (DMA issue latency + flight),
            # so fill that window with junk matmuls on a zeroed tile; the real
            # matmul stream then starts at the full 2.4 GHz.
            warm = raw_pool.tile([128, 256], u8, tag="warm")
            ps_w = psum_pool.tile([128, 512], f32)
            nc.gpsimd.memset(warm[:], 0)
            for i in range(N_WARM):
                nc.tensor.matmul(
                    ps_w[:, :256],
                    warm[:, :128].bitcast(f8),
                    warm[:].bitcast(f8),
                    start=True,
                    stop=True,
                )

            # emit per piece: DMA -> mask-AND -> ab matmuls -> mm matmuls.
            # The last two pieces' ab matmuls are hoisted before their mm
            # matmuls so ps_ab's accumulation stops ~1.3us before ps_mm's and
            # its copy+output DMA hide under the mm tail.
            deferred_mm = []
            for i, (g, c0, c1) in enumerate(_PIECES):
                a_w, _, w = _piece_geom(c0, c1)
                t = raw_pool.tile([128, 2, w], u8, tag=f"t{w}")
                m = raw_pool.tile([128, 2, w], u8, tag=f"m{w}")
                eng = nc.sync if i % 2 == 0 else nc.scalar
                eng.dma_start(t[:], piece_d[i][:])
                nc.vector.tensor_scalar(
                    m[:].bitcast(u16),
                    t[:].bitcast(u16),
                    0x0101,
                    None,
                    op0=Alu.bitwise_and,
                )
                mms = []
                for c in range(c0, c1 + 1):
                    a0 = (c - c0) * 128
                    b0 = a_w + a0
                    first = i == 0 and c == c0
                    last = i == len(_PIECES) - 1 and c == c1
                    nc.tensor.matmul(
                        ps_ab[:],
                        t[:, :, a0 : a0 + 128].bitcast(f8),
                        t[:, :, b0 : b0 + NW].bitcast(f8),
                        start=first,
                        stop=last,
                        perf_mode=DR,
                    )
                    mms.append(
                        (m, a0, b0, first, last)
                    )
                if i >= len(_PIECES) - 2:
                    deferred_mm.extend(mms)
                else:
                    for m_, a0, b0, first, last in mms:
                        nc.tensor.matmul(
                            ps_mm[:],
                            m_[:, :, a0 : a0 + 128].bitcast(f8),
                            m_[:, :, b0 : b0 + NW].bitcast(f8),
                            start=first,
                            stop=last,
                            perf_mode=DR,
                        )

            out_sb = raw_pool.tile([128, 2, NW], f32, tag="out")
            nc.vector.tensor_copy(out_sb[:, 0], ps_ab[:])
            nc.sync.dma_start(out_d[:, 0], out_sb[:, 0])
            for m_, a0, b0, first, last in deferred_mm:
                nc.tensor.matmul(
                    ps_mm[:],
                    m_[:, :, a0 : a0 + 128].bitcast(f8),
                    m_[:, :, b0 : b0 + NW].bitcast(f8),
                    start=first,
                    stop=last,
                    perf_mode=DR,
                )
            nc.vector.tensor_copy(out_sb[:, 1], ps_mm[:])
            nc.scalar.dma_start(out_d[:, 1], out_sb[:, 1])

    import bass_rust as _bass_rust

    _bass_rust.move_matmul_waits_to_ldweights(nc.m)
    _bass_rust.generate_event_semaphores(nc)
    return nc


def _get_program():
    if "nc" not in _CACHE:
        _CACHE["nc"] = _build_program()
    return _CACHE["nc"]


def _encode(iris, mask):
    """(2,B,L,2) bool pair -> (2*B, 2, L) uint8 (ia<<7)|ma, k de-interleaved."""
    enc = (iris.astype(np.uint8) << 7) | mask.astype(np.uint8)
    # (2, B, L, 2) -> (2, B, 2, L) -> (2*B, 2, L)
    return enc.transpose(0, 1, 3, 2).reshape(2 * B_FULL, 2, L)


def kernel(iris_codes_a, mask_codes_a, iris_codes_b, mask_codes_b, _trace=False):
    from concourse.bass_utils import run_bass_kernel_spmd

    nc = _get_program()

    a_full = _encode(np.asarray(iris_codes_a), np.asarray(mask_codes_a))
    b_enc = _encode(np.asarray(iris_codes_b), np.asarray(mask_codes_b))
    # circular halo of +-R on the plane axis
    b_full = np.concatenate(
        [b_enc[:, :, L - R :], b_enc, b_enc[:, :, :R]], axis=2
    )

    def rows(c):
        # rows of core c: eyes i in {0,1} x batches [c*B_SH, (c+1)*B_SH)
        return np.r_[
            c * B_SH : (c + 1) * B_SH, B_FULL + c * B_SH : B_FULL + (c + 1) * B_SH
        ]

    in_maps = []
    for c in range(N_CORES):
        a_c = a_full[rows(c)]
        b_c = b_full[rows(c)]
        im = {}
        for i, (g, c0, c1) in enumerate(_PIECES):
            a_w, b_w, w = _piece_geom(c0, c1)
            rs = slice(g * 128, (g + 1) * 128)
            p = np.zeros((128, 2, w), np.uint8)
            p[:, :, :a_w] = a_c[rs, :, c0 * 128 : c0 * 128 + a_w]
            p[:, :, a_w : a_w + b_w] = b_c[rs, :, c0 * 128 : c0 * 128 + b_w]
            im[f"p{i}"] = p
        in_maps.append(im)
    res = run_bass_kernel_spmd(nc, in_maps, list(range(N_CORES)), trace=_trace)
    _CACHE["last_result"] = res

    acc = np.zeros((128, 2, NW), np.float64)
    for r in res.results:
        acc += r["out"].astype(np.float64)

    shifts = np.arange(-R, R + 1)
    cab = np.array([np.trace(acc[:, 0], offset=R + s) for s in shifts])
    den = np.array([np.trace(acc[:, 1], offset=R + s) for s in shifts])
    cab = np.rint(cab * 2.0**18)
    den = np.rint(den * 2.0**18)
    num = (den - cab) / 2.0
    dist = num.astype(np.float32) / den.astype(np.float32)
    out = np.minimum(np.float32(1.0), dist.min())
    return np.asarray([out], dtype=np.float32)


# revision 7
# speedup vs baseline: 1.1687x; 1.1687x over previous
"""Masked fractional Hamming distance over 31 circular rotations, on 8 trn2 cores.

Math: for shift s, num(s)/den(s) with
  den(s) = sum_{t,k} ma_k[t] * mb_k[t+s]          (correlation at lag s)
  num(s) = masked differing bits; with the sign-encode
  A = (ia<<7)|ma, B = (ib<<7)|mb read as fp8e4m3 the bytes become
  {+0, -0, +2^-9, -2^-9} (sign=iris, magnitude=mask), so
  corr(A, B)(s) = (den - 2*num) * 2^-18, corr(A&1, B&1)(s) = den * 2^-18.

The encode happens on the HOST (halves HBM traffic: 2 tensors instead of 4);
the two k-planes are de-interleaved on the host and summed inside the PE via
fp8 DoubleRow matmuls (lhsT [K,2,128], rhs [K,2,158] -> psum [128,158] with
result = sum_i W_i.T @ X_i at 2x fp8 rate). The +-15 circular halo is baked
into B on the host. Masks (byte&1) are extracted on-device with one u16 AND.
Each 128-row group is split into two DMA pieces (chunk 0-8 / 9-15 windows)
so compute chases the DMA stream at half-group granularity; all pieces are
prefetched (bufs=8).

A short junk-matmul chain at kernel start keeps the PE busy while the first
piece is in flight so the HAM clock gate reaches 2.4 GHz before the real
matmul stream begins.  Within a piece all ab matmuls run before the mm
matmuls (the mask-AND only gates the mm half), and the last two pieces' mm
matmuls are deferred so ps_ab's accumulation stops ~1.3us early and its
psum copy + output DMA hide under the mm tail; the second output half goes
out on the Activation DGE queue so the two output DMAs don't serialize.
Band diagonals are summed on the host (exact integers scaled by 2^-18).
"""

import numpy as np

N_CORES = 8
B_FULL, L = 4096, 2048
R = 15
B_SH = B_FULL // N_CORES       # 512 batches per core
ROWS = 2 * B_SH                # 1024 rows per core (2 eyes x 512 batches)
NW = 128 + 2 * R               # 158 moving window
LH = L + 2 * R                 # 2078 halo-padded plane length
N_GROUPS = ROWS // 128         # 8
N_CHUNKS = L // 128            # 16
C_SPLIT = 9                    # chunks 0-8 from the lo piece, 9-15 from hi
A_LO, B_LO = C_SPLIT * 128, C_SPLIT * 128 + 2 * R     # 1152, 1182
A_HI, B_HI = L - A_LO, LH - A_LO                      # 896, 926
# plane stride padded to a multiple of 16 (ldweights needs aligned strides)
W_LO = -(-(A_LO + B_LO) // 16) * 16                   # 2336
W_HI = -(-(A_HI + B_HI) // 16) * 16                   # 1824
N_WARM = 14                    # junk matmuls to warm the PE clock gate

_CACHE = {}


def _build_program():
    import concourse.bass as bass
    import concourse.tile as tile
    from concourse import mybir

    u8 = mybir.dt.uint8
    u16 = mybir.dt.uint16
    f8 = mybir.dt.float8e4
    f32 = mybir.dt.float32
    Alu = mybir.AluOpType
    DR = mybir.MatmulPerfMode.DoubleRow

    nc = bass.Bass()
    lo_d = nc.declare_dram_parameter("lo", [N_GROUPS, 128, 2, W_LO], u8, isOutput=False)
    hi_d = nc.declare_dram_parameter("hi", [N_GROUPS, 128, 2, W_HI], u8, isOutput=False)
    out_d = nc.declare_dram_parameter("out", [128, 2, NW], f32, isOutput=True)

    with tile.TileContext(nc) as tc:
        with (
            tc.tile_pool(name="raw", bufs=8) as raw_pool,
            tc.tile_pool(name="acc", bufs=1, space="PSUM") as psum_pool,
        ):
            ps_ab = psum_pool.tile([128, NW], f32)
            ps_mm = psum_pool.tile([128, NW], f32)

            # PE warm-up against the HAM clock gate (see module docstring).
            warm = raw_pool.tile([128, 256], u8, tag="warm")
            ps_w = psum_pool.tile([128, 512], f32)
            nc.gpsimd.memset(warm[:], 0)
            for i in range(N_WARM):
                nc.tensor.matmul(
                    ps_w[:, :256],
                    warm[:, :128].bitcast(f8),
                    warm[:].bitcast(f8),
                    start=True,
                    stop=True,
                )

            n_pieces = 2 * N_GROUPS
            deferred_mm = []
            piece = 0
            for g in range(N_GROUPS):
                for half, (dram, width, a_w) in enumerate(
                    ((lo_d, W_LO, A_LO), (hi_d, W_HI, A_HI))
                ):
                    t = raw_pool.tile([128, 2, width], u8, tag=f"t{half}")
                    m = raw_pool.tile([128, 2, width], u8, tag=f"m{half}")
                    nc.sync.dma_start(t[:], dram[g])
                    nc.vector.tensor_scalar(
                        m[:].bitcast(u16),
                        t[:].bitcast(u16),
                        0x0101,
                        None,
                        op0=Alu.bitwise_and,
                    )
                    c_range = range(C_SPLIT) if half == 0 else range(C_SPLIT, N_CHUNKS)
                    mms = []
                    for c in c_range:
                        a0 = c * 128 - (0 if half == 0 else A_LO)
                        b0 = a_w + a0
                        first = g == 0 and c == 0
                        last = g == N_GROUPS - 1 and c == N_CHUNKS - 1
                        nc.tensor.matmul(
                            ps_ab[:],
                            t[:, :, a0 : a0 + 128].bitcast(f8),
                            t[:, :, b0 : b0 + NW].bitcast(f8),
                            start=first,
                            stop=last,
                            perf_mode=DR,
                        )
                        mms.append((m, a0, b0, first, last))
                    if piece >= n_pieces - 2:
                        deferred_mm.extend(mms)
                    else:
                        for m_, a0, b0, first, last in mms:
                            nc.tensor.matmul(
                                ps_mm[:],
                                m_[:, :, a0 : a0 + 128].bitcast(f8),
                                m_[:, :, b0 : b0 + NW].bitcast(f8),
                                start=first,
                                stop=last,
                                perf_mode=DR,
                            )
                    piece += 1

            out_sb = raw_pool.tile([128, 2, NW], f32, tag="out")
            nc.vector.tensor_copy(out_sb[:, 0], ps_ab[:])
            nc.sync.dma_start(out_d[:, 0], out_sb[:, 0])
            for m_, a0, b0, first, last in deferred_mm:
                nc.tensor.matmul(
                    ps_mm[:],
                    m_[:, :, a0 : a0 + 128].bitcast(f8),
                    m_[:, :, b0 : b0 + NW].bitcast(f8),
                    start=first,
                    stop=last,
                    perf_mode=DR,
                )
            nc.vector.tensor_copy(out_sb[:, 1], ps_mm[:])
            nc.scalar.dma_start(out_d[:, 1], out_sb[:, 1])

    import bass_rust as _bass_rust

    _bass_rust.move_matmul_waits_to_ldweights(nc.m)
    _bass_rust.generate_event_semaphores(nc)
    return nc


def _get_program():
    if "nc" not in _CACHE:
        _CACHE["nc"] = _build_program()
    return _CACHE["nc"]


def _encode(iris, mask):
    """(2,B,L,2) bool pair -> (2*B, 2, L) uint8 (ia<<7)|ma, k de-interleaved."""
    enc = (iris.astype(np.uint8) << 7) | mask.astype(np.uint8)
    # (2, B, L, 2) -> (2, B, 2, L) -> (2*B, 2, L)
    return enc.transpose(0, 1, 3, 2).reshape(2 * B_FULL, 2, L)


def kernel(iris_codes_a, mask_codes_a, iris_codes_b, mask_codes_b, _trace=False):
    from concourse.bass_utils import run_bass_kernel_spmd

    nc = _get_program()

    a_full = _encode(np.asarray(iris_codes_a), np.asarray(mask_codes_a))
    b_enc = _encode(np.asarray(iris_codes_b), np.asarray(mask_codes_b))
    # circular halo of +-R on the plane axis
    b_full = np.concatenate(
        [b_enc[:, :, L - R :], b_enc, b_enc[:, :, :R]], axis=2
    )

    def rows(c):
        # rows of core c: eyes i in {0,1} x batches [c*B_SH, (c+1)*B_SH)
        return np.r_[
            c * B_SH : (c + 1) * B_SH, B_FULL + c * B_SH : B_FULL + (c + 1) * B_SH
        ]

    in_maps = []
    for c in range(N_CORES):
        a_c = a_full[rows(c)]
        b_c = b_full[rows(c)]
        lo = np.zeros((ROWS, 2, W_LO), np.uint8)
        hi = np.zeros((ROWS, 2, W_HI), np.uint8)
        lo[:, :, :A_LO] = a_c[:, :, :A_LO]
        lo[:, :, A_LO : A_LO + B_LO] = b_c[:, :, :B_LO]
        hi[:, :, :A_HI] = a_c[:, :, A_LO:]
        hi[:, :, A_HI : A_HI + B_HI] = b_c[:, :, A_LO:]
        in_maps.append(
            {
                "lo": lo.reshape(N_GROUPS, 128, 2, W_LO),
                "hi": hi.reshape(N_GROUPS, 128, 2, W_HI),
            }
        )
    res = run_bass_kernel_spmd(nc, in_maps, list(range(N_CORES)), trace=_trace)
    _CACHE["last_result"] = res

    acc = np.zeros((128, 2, NW), np.float64)
    for r in res.results:
        acc += r["out"].astype(np.float64)

    shifts = np.arange(-R, R + 1)
    cab = np.array([np.trace(acc[:, 0], offset=R + s) for s in shifts])
    den = np.array([np.trace(acc[:, 1], offset=R + s) for s in shifts])
    cab = np.rint(cab * 2.0**18)
    den = np.rint(den * 2.0**18)
    num = (den - cab) / 2.0
    dist = num.astype(np.float32) / den.astype(np.float32)
    out = np.minimum(np.float32(1.0), dist.min())
    return np.asarray([out], dtype=np.float32)
